# revision 18
# baseline (speedup 1.0000x reference)
"""Bass/Trainium2 kernel for windowed multi-head attention (8 NeuronCores).

Math per window b (64 tokens), matching torch-style nn.MultiHeadAttention:
  qh = (q @ Wq.T + bq) * scale; kh = k @ Wk.T + bk; vh = v @ Wv.T + bv
  S  = qh @ kh.T  (per head);  S[masked k] = -1e4;  P = softmax(S)
  out = concat_h(P @ vh) @ Wp.T + bp

Device dataflow (per core, 256 windows = 16384 tokens; f32 PSUM accum):
  - q,k are fed as fp8(e4m3) feature-major [128, 4, TT] T-tile blocks and
    projected with fp8 DoubleRow matmuls (K=256 per MM); the fp8 weight
    pre-scales (SQ/SK, needed to clear e4m3's subnormal cutoff) are folded
    back out in the PSUM->SBUF copies. v stays bf16.
  - The mask never touches S. Instead masked softmax is computed as
       x = exp(S) @ (m * vh) / (exp(S) @ m)
    i.e. vh rows are zeroed during the v PSUM->SBUF copy (per-partition
    scale), and per-head denominators come from a K=128 matmul of exp(S).T
    against a masked parity selector.
  - S is computed TRANSPOSED from the start: lhsT = zero-padded per-head-
    slot k layout (block-diag over the 2 heads sharing a 128-feature
    chunk), rhs = feature-major qh. The result spT[(parity,tk), ch, w, tq]
    feeds exp directly in the layout the P@V matmuls consume - no PE
    transposes, no separate P normalize. Normalization is applied after
    P@V: xn = xt * recT where recT is a K=2 parity-broadcast matmul of the
    reciprocal denominators.
  - Per-pair work is software-pipelined (stage A = S+exp for pair p+1
    issued before stage B = denominators/PV/out-proj of pair p) so the PE
    queue never stalls on ACT/DVE round trips.
  - PSUM->SBUF copy work is split across ACT (k zero-pad copies, masked v
    copies, exp) and DVE (q copies, reciprocal, normalize, out copies).

All matmul operands sit at partition base 0 with K in {2, 128} or K=64 at
base 0: mixing row-group offsets across back-to-back K<128 matmuls crashes
the PE.

Biases are zero in this problem; nonzero bv/bp fold into a host-side
output add, nonzero bq/bk fall back to a host computation.
"""

import sys

for _p in ("/opt/trn_rl_repo",):
    if _p not in sys.path:
        sys.path.append(_p)

import numpy as np
import ml_dtypes

import concourse.bacc as bacc
import concourse.bass as bass
import concourse.mybir as mybir
import concourse.tile as tile
from concourse.bass import ds, ts
from concourse.bass_utils import run_bass_kernel_spmd

BF16 = mybir.dt.bfloat16
F32 = mybir.dt.float32
FP8 = mybir.dt.float8e4
NP_BF16 = ml_dtypes.bfloat16
NP_FP8 = ml_dtypes.float8_e4m3  # TRN FP8_EXP4: max +-240, inf beyond

NCORES = 8
DIM = 512
HEADS = 8
HD = 64
N = 64  # window length
B_TOTAL = 2048
WIN_PER_CORE = B_TOTAL // NCORES  # 256
TOK_PER_CORE = WIN_PER_CORE * N  # 16384
SCALE = HD ** -0.5
TT = 512  # token tile (8 windows, 4 pairs)
PAIRS = TT // 128  # 4
# fp8 weight pre-scales (keep e4m3 values out of the subnormal range);
# folded back out in the PSUM->SBUF copy after each projection.
SQ = 128.0
SK = 16.0


def build_program(win_per_core=WIN_PER_CORE):
    """Build the per-core Tile program. All 8 cores run it SPMD."""
    tok = win_per_core * N
    n_tt = tok // TT

    nc = bacc.Bacc("TRN2", debug=False)

    qT_d = nc.dram_tensor("qT", [n_tt, 128, 4, TT], FP8, kind="ExternalInput")
    kT_d = nc.dram_tensor("kT", [n_tt, 128, 4, TT], FP8, kind="ExternalInput")
    vT_d = nc.dram_tensor("vT", [n_tt, 128, 4, TT], BF16, kind="ExternalInput")
    # msel[p, pr, w, j] = mask01 of window (pr,w) token p%64 if p//64==j else 0
    msel_d = nc.dram_tensor(
        "msel", [n_tt, 128, PAIRS, 2, 2], BF16, kind="ExternalInput"
    )
    # m01[p, ct] = mask01 of T-tile token ct*128+p (v-row masking)
    m01_d = nc.dram_tensor("m01", [n_tt, 128, PAIRS], F32, kind="ExternalInput")
    # fp8 DoubleRow weights: [ki 128, kpair 2, ko 2, fo 512], input feature
    # f = (2*kp + ko)*128 + ki
    wq = nc.dram_tensor("wq", [128, 2, 2, DIM], FP8, kind="ExternalInput")
    wk = nc.dram_tensor("wk", [128, 2, 2, DIM], FP8, kind="ExternalInput")
    wv = nc.dram_tensor("wv", [DIM, DIM], BF16, kind="ExternalInput")
    wp = nc.dram_tensor("wp", [DIM, DIM], BF16, kind="ExternalInput")
    sel01 = nc.dram_tensor("sel01", [2, 128], BF16, kind="ExternalInput")
    out = nc.dram_tensor("out", [tok, DIM], BF16, kind="ExternalOutput")

    qa, ka, va = qT_d.ap(), kT_d.ap(), vT_d.ap()
    oa = out.ap()

    with tile.TileContext(nc) as tc:
        with (
            tc.tile_pool(name="consts", bufs=1) as consts,
            tc.tile_pool(name="pin", bufs=3) as pin,
            tc.tile_pool(name="py", bufs=2) as py,
            tc.tile_pool(name="pp", bufs=3) as pp,
            tc.tile_pool(name="pst", bufs=2) as pst,
            tc.tile_pool(name="pout", bufs=4) as pout,
            tc.tile_pool(name="ps_pj", bufs=2, space="PSUM") as ps_pj,
            tc.tile_pool(name="ps_s", bufs=2, space="PSUM") as ps_s,
            tc.tile_pool(name="ps_dn", bufs=1, space="PSUM") as ps_dn,
            tc.tile_pool(name="ps_rt", bufs=1, space="PSUM") as ps_rt,
            tc.tile_pool(name="ps_x", bufs=1, space="PSUM") as ps_x,
            tc.tile_pool(name="ps_o", bufs=1, space="PSUM") as ps_o,
        ):
            # Weights: fp8 DoubleRow layout for q/k, bf16 [fi_part 128,
            # fi_chunk 4, fo 512] for v/p.
            w_tiles = {}
            for name, t in (("wq", wq), ("wk", wk)):
                wt = consts.tile([128, 2, 2, DIM], FP8, tag=f"w_{name}")
                nc.sync.dma_start(out=wt, in_=t.ap())
                w_tiles[name] = wt
            for name, t in (("wv", wv), ("wp", wp)):
                wt = consts.tile([128, 4, DIM], BF16, tag=f"w_{name}")
                nc.sync.dma_start(
                    out=wt, in_=t.ap().rearrange("(c p) f -> p c f", p=128)
                )
                w_tiles[name] = wt
            sel_t = consts.tile([2, 128], BF16, tag="sel01")
            nc.sync.dma_start(out=sel_t, in_=sel01.ap())

            for tt_i in range(n_tt):
                t0 = tt_i * TT
                # ---- transposed input loads (host pre-transposed) ----
                qT = pin.tile([128, 4, TT], FP8, tag="qT")
                kT = pin.tile([128, 4, TT], FP8, tag="kT")
                vT = pin.tile([128, 4, TT], BF16, tag="vT")
                nc.sync.dma_start(out=qT, in_=qa[tt_i])
                nc.sync.dma_start(out=kT, in_=ka[tt_i])
                nc.sync.dma_start(out=vT, in_=va[tt_i])
                msel_t = pin.tile([128, PAIRS, 2, 2], BF16, tag="msel")
                nc.sync.dma_start(out=msel_t, in_=msel_d.ap()[tt_i])
                m01_t = pin.tile([128, PAIRS], F32, tag="m01")
                nc.sync.dma_start(out=m01_t, in_=m01_d.ap()[tt_i])

                # ---- q projection (fp8 DoubleRow, K=256 per MM) ----
                # -> feature-major bf16 [fo 128, c, tok]; copies on DVE,
                # 1/SQ undoes the fp8 weight pre-scale.
                yqT = py.tile([128, 4, TT], BF16, tag="yqT")
                for co in range(4):
                    ps = ps_pj.tile([128, TT], F32, tag="pj")
                    for kp in range(2):
                        nc.tensor.matmul(
                            ps,
                            lhsT=w_tiles["wq"][:, kp, :, ts(co, 128)],
                            rhs=qT[:, ds(2 * kp, 2), :],
                            start=(kp == 0),
                            stop=(kp == 1),
                            perf_mode=mybir.MatmulPerfMode.DoubleRow,
                        )
                    nc.vector.tensor_scalar_mul(yqT[:, co, :], ps, 1.0 / SQ)

                # ---- k projection -> zero-padded per-head-slot layout ----
                # ykTz[p<64, c, pr, w, 0, tk] = head-2c rows, ykTz[p>=64, c,
                # pr, w, 1, tk] = head-2c+1 rows, other slots zero. The
                # (slot, tk) tail is contiguous, so ykTz[:, c, pr, w] is the
                # block-diag stationary operand of an S.T matmul. Zero slots
                # are never overwritten, so memset only on each pool
                # buffer's first use.
                ykTz = py.tile([128, 4, PAIRS, 2, 2, N], BF16, tag="ykTz")
                if tt_i < 2:
                    nc.gpsimd.memset(ykTz[0:64, :, :, :, 1, :], 0.0)
                    nc.gpsimd.memset(ykTz[64:128, :, :, :, 0, :], 0.0)
                for co in range(4):
                    ps = ps_pj.tile([128, TT], F32, tag="pj")
                    for kp in range(2):
                        nc.tensor.matmul(
                            ps,
                            lhsT=w_tiles["wk"][:, kp, :, ts(co, 128)],
                            rhs=kT[:, ds(2 * kp, 2), :],
                            start=(kp == 0),
                            stop=(kp == 1),
                            perf_mode=mybir.MatmulPerfMode.DoubleRow,
                        )
                    psw = ps.rearrange("p (r w t) -> p r w t", r=PAIRS, w=2)
                    nc.scalar.mul(
                        out=ykTz[0:64, co, :, :, 0, :],
                        in_=psw[0:64],
                        mul=1.0 / SK,
                    )
                    nc.scalar.mul(
                        out=ykTz[64:128, co, :, :, 1, :],
                        in_=psw[64:128],
                        mul=1.0 / SK,
                    )

                # ---- v projection -> masked token-major bf16 ----
                # vh rows of masked tokens are zeroed via the per-partition
                # copy scale (numerator side of the masked softmax).
                yv = py.tile([128, PAIRS, DIM], BF16, tag="yv")
                for ct in range(PAIRS):
                    ps = ps_pj.tile([128, DIM], F32, tag="pj")
                    for ci in range(4):
                        nc.tensor.matmul(
                            ps,
                            lhsT=vT[:, ci, ts(ct, 128)],
                            rhs=w_tiles["wv"][:, ci, :],
                            start=(ci == 0),
                            stop=(ci == 3),
                        )
                    nc.scalar.activation(
                        out=yv[:, ct, :],
                        in_=ps,
                        func=mybir.ActivationFunctionType.Copy,
                        scale=m01_t[:, ds(ct, 1)],
                    )
                # second window's vh rows relocated to partition base 0
                yv_hi = py.tile([64, PAIRS, DIM], BF16, tag="yv_hi")
                nc.gpsimd.dma_start(out=yv_hi, in_=yv[64:128, :, :])

                # ---- attention, software-pipelined over 128-token pairs ----
                def stage_a(pr):
                    """S.T for all 8 heads + exp + odd-parity relocation."""
                    base = pr * 128
                    spT = ps_s.tile([128, 4, 2, N], F32, tag="spT")
                    for ch in range(4):
                        for w in range(2):
                            nc.tensor.matmul(
                                spT[:, ch, w, :],
                                lhsT=ykTz[:, ch, pr, w, :, :],
                                rhs=yqT[:, ch, ds(base + 64 * w, 64)],
                                start=True,
                                stop=True,
                            )
                    pexpT = pp.tile([128, 4, 2, N], BF16, tag="pexpT")
                    nc.scalar.activation(
                        out=pexpT, in_=spT, func=mybir.ActivationFunctionType.Exp
                    )
                    return (pexpT,)

                def stage_b1(pr, pexpT):
                    """Denominators + normalized P.T (pnT)."""
                    # dn[j, ch, w, tq] = sum over parity-j partitions of
                    # msel * pexpT = per-head masked softmax denominators
                    dn = ps_dn.tile([2, 4, 2, N], F32, tag="dn")
                    for w in range(2):
                        nc.tensor.matmul(
                            dn[:, :, w, :],
                            lhsT=msel_t[:, pr, w, :],
                            rhs=pexpT[:, :, w, :],
                            start=True,
                            stop=True,
                        )
                    rec = pst.tile([2, 4, 2, N], BF16, tag="rec")
                    with nc.allow_low_precision("bf16 softmax denominators"):
                        nc.vector.reciprocal(out=rec, in_=dn)
                    # broadcast rec to the 64-partition parity blocks
                    rT = ps_rt.tile([128, 4, 2, N], F32, tag="rT")
                    nc.tensor.matmul(rT, lhsT=sel_t, rhs=rec, start=True, stop=True)
                    pnT = pp.tile([128, 4, 2, N], BF16, tag="pnT")
                    nc.vector.tensor_tensor(
                        out=pnT, in0=pexpT, in1=rT, op=mybir.AluOpType.mult
                    )
                    pnT_od = pp.tile([64, 4, 2, N], BF16, tag="pnT_od")
                    nc.gpsimd.dma_start(out=pnT_od, in_=pnT[64:128, :, :, :])
                    return pnT, pnT_od

                def stage_b2(pr, pnT, pnT_od):
                    """P@V and output projection."""
                    base = pr * 128
                    # X.T: [feat-in-chunk 128, ch, w, tq]
                    xt = ps_x.tile([128, 4, 2, N], F32, tag="xt")
                    for h in range(HEADS):
                        ch, hh = h // 2, (h % 2) * 64
                        psrc = pnT if h % 2 == 0 else pnT_od
                        for w in range(2):
                            vsrc = yv if w == 0 else yv_hi
                            nc.tensor.matmul(
                                xt[ds(hh, 64), ch, w, :],
                                lhsT=vsrc[ds(0, 64), pr, ts(h, 64)],
                                rhs=psrc[ds(0, 64), ch, w, :],
                                start=True,
                                stop=True,
                            )
                    xts = pp.tile([128, 4, 2, N], BF16, tag="xts")
                    nc.vector.tensor_copy(out=xts, in_=xt)
                    # output projection: [tok 128, fo 512]
                    po = ps_o.tile([128, DIM], F32, tag="po")
                    for c in range(4):
                        nc.tensor.matmul(
                            po,
                            lhsT=xts[:, c, :, :],
                            rhs=w_tiles["wp"][:, c, :],
                            start=(c == 0),
                            stop=(c == 3),
                        )
                    osb = pout.tile([128, DIM], BF16, tag="osb")
                    nc.vector.tensor_copy(out=osb, in_=po)
                    nc.sync.dma_start(
                        out=oa[t0 + base : t0 + base + 128, :], in_=osb
                    )

                pa = [None] * PAIRS
                pb = [None] * PAIRS
                pa[0] = stage_a(0)
                pa[1] = stage_a(1)
                pb[0] = stage_b1(0, *pa[0])
                pa[2] = stage_a(2)
                stage_b2(0, *pb[0])
                pb[1] = stage_b1(1, *pa[1])
                pa[3] = stage_a(3)
                stage_b2(1, *pb[1])
                pb[2] = stage_b1(2, *pa[2])
                stage_b2(2, *pb[2])
                pb[3] = stage_b1(3, *pa[3])
                stage_b2(3, *pb[3])

    nc.compile()
    return nc


_PROGRAM_CACHE = {}


def _get_program(win_per_core):
    if win_per_core not in _PROGRAM_CACHE:
        _PROGRAM_CACHE[win_per_core] = build_program(win_per_core)
    return _PROGRAM_CACHE[win_per_core]


def _feature_major_tiles(x_flat):
    """[tok, 512] -> [n_tt, 128, 4, TT] so each T-tile block is one
    fully-contiguous transposed DMA read."""
    tok = x_flat.shape[0]
    n_tt = tok // TT
    xt = x_flat.reshape(n_tt, TT, 4, 128).transpose(0, 3, 2, 1)
    return np.ascontiguousarray(xt)


def _fp8(x):
    return np.clip(np.asarray(x, np.float32), -240.0, 240.0).astype(NP_FP8)


def _dr_weights(wT_scaled):
    """[512 fi, 512 fo] -> DoubleRow fp8 layout [ki 128, kp 2, ko 2, fo]."""
    w8 = _fp8(wT_scaled)
    return np.ascontiguousarray(
        w8.reshape(2, 2, 128, DIM).transpose(2, 0, 1, 3)
    )


def make_in_maps(q, k, v, mask, Wq, Wk, Wv, Wp, ncores=NCORES):
    """Host-side shard + layout prep. Returns list of per-core input dicts."""
    B, n, C = q.shape
    win_pc = B // ncores
    qf = np.ascontiguousarray(q.reshape(B * n, C))
    kf = np.ascontiguousarray(k.reshape(B * n, C))
    vf = np.ascontiguousarray(v.reshape(B * n, C))

    mflat = (mask != 0).astype(np.float32).reshape(B * n)

    wq_t = _dr_weights(Wq.T * (SCALE * SQ))
    wk_t = _dr_weights(Wk.T * SK)
    wv_t = np.ascontiguousarray(Wv.T.astype(NP_BF16))
    wp_t = np.ascontiguousarray(Wp.T.astype(NP_BF16))
    sel = np.zeros((2, 128), NP_BF16)
    sel[0, 0:64] = 1
    sel[1, 64:128] = 1

    tok_pc = win_pc * n
    n_tt = tok_pc // TT
    in_maps = []
    for c in range(ncores):
        sl = slice(c * tok_pc, (c + 1) * tok_pc)
        mc = mflat[sl]
        # m01[tt, p, ct] = mask of token tt*512 + ct*128 + p
        m01 = np.ascontiguousarray(
            mc.reshape(n_tt, PAIRS, 128).transpose(0, 2, 1)
        )
        # msel[tt, p, pr, w, j]: parity-j selector rows carry the window
        # (pr, w) mask at tk = p % 64
        mwin = mc.reshape(n_tt, PAIRS, 2, 64).transpose(0, 3, 1, 2)
        msel = np.zeros((n_tt, 128, PAIRS, 2, 2), NP_BF16)
        msel[:, 0:64, :, :, 0] = mwin
        msel[:, 64:128, :, :, 1] = mwin
        in_maps.append(
            {
                "qT": _feature_major_tiles(_fp8(qf[sl])),
                "kT": _feature_major_tiles(_fp8(kf[sl])),
                "vT": _feature_major_tiles(vf[sl].astype(NP_BF16)),
                "msel": msel,
                "m01": m01,
                "wq": wq_t,
                "wk": wk_t,
                "wv": wv_t,
                "wp": wp_t,
                "sel01": sel,
            }
        )
    return in_maps


def _reference_numpy(q, k, v, mask, Wq, bq, Wk, bk, Wv, bv, Wp, bp):
    """Full-precision host fallback (only used for nonzero bq/bk)."""
    B, n, C = q.shape
    qh = (q.reshape(-1, C) @ Wq.T + bq).reshape(B, n, HEADS, HD).transpose(0, 2, 1, 3)
    kh = (k.reshape(-1, C) @ Wk.T + bk).reshape(B, n, HEADS, HD).transpose(0, 2, 1, 3)
    vh = (v.reshape(-1, C) @ Wv.T + bv).reshape(B, n, HEADS, HD).transpose(0, 2, 1, 3)
    s = np.einsum("bhqd,bhkd->bhqk", qh * SCALE, kh)
    s = np.where((mask[:, None, None, :] == 0), np.float32(-10000.0), s)
    s = s - s.max(-1, keepdims=True)
    e = np.exp(s)
    p = e / e.sum(-1, keepdims=True)
    x = np.einsum("bhqk,bhkd->bhqd", p, vh)
    x = x.transpose(0, 2, 1, 3).reshape(B, n, C)
    return (x @ Wp.T + bp).astype(np.float32)


def kernel(q, k, v, mask, Wq, bq, Wk, bk, Wv, bv, Wp, bp, trace=False):
    q = np.asarray(q, np.float32)
    k = np.asarray(k, np.float32)
    v = np.asarray(v, np.float32)
    mask = np.asarray(mask)
    Wq, Wk, Wv, Wp = (np.asarray(w, np.float32) for w in (Wq, Wk, Wv, Wp))
    bq, bk, bv, bp = (np.asarray(b, np.float32) for b in (bq, bk, bv, bp))

    if np.any(bq) or np.any(bk):
        return _reference_numpy(q, k, v, mask, Wq, bq, Wk, bk, Wv, bv, Wp, bp)

    B, n, C = q.shape
    win_pc = B // NCORES
    nc = _get_program(win_pc)
    in_maps = make_in_maps(q, k, v, mask, Wq, Wk, Wv, Wp)
    res = run_bass_kernel_spmd(
        nc, in_maps, core_ids=list(range(NCORES)), trace=trace
    )
    outs = np.concatenate(
        [np.asarray(r["out"], np.float32) for r in res.results], axis=0
    )
    outs = outs.reshape(B, n, C)
    # bv flows through attention linearly (softmax rows sum to 1); with bp it
    # folds into a single output bias.
    bout = bp + bv @ Wp.T
    if np.any(bout):
        outs = outs + bout.astype(np.float32)
    if trace:
        kernel._last_result = res
    return outs


# revision 27
# speedup vs baseline: 1.2920x; 1.2920x over previous
"""Bass/Trainium2 kernel for windowed multi-head attention (8 NeuronCores).

Math per window b (64 tokens), matching torch-style nn.MultiHeadAttention:
  qh = (q @ Wq.T + bq) * scale; kh = k @ Wk.T + bk; vh = v @ Wv.T + bv
  S  = qh @ kh.T  (per head);  S[masked k] = -1e4;  P = softmax(S)
  out = concat_h(P @ vh) @ Wp.T + bp

Device dataflow (per core, 256 windows = 16384 tokens; f32 PSUM accum):
  - q,k are fed as fp8(e4m3) feature-major [128, 4, TT] T-tile blocks and
    projected with fp8 DoubleRow matmuls (K=256 per MM); the fp8 weight
    pre-scales (SQ/SK, needed to clear e4m3's subnormal cutoff) are folded
    back out in the PSUM->SBUF copies. v stays bf16.
  - The mask never touches S. Instead masked softmax is computed as
       x = exp(S) @ (m * vh) / (exp(S) @ m)
    i.e. vh rows are zeroed during the v PSUM->SBUF copy (per-partition
    scale), and per-head denominators come from a K=128 matmul of exp(S).T
    against a masked parity selector.
  - S is computed TRANSPOSED from the start: lhsT = zero-padded per-head-
    slot k layout (block-diag over the 2 heads sharing a 128-feature
    chunk), rhs = feature-major qh. The result spT[(parity,tk), ch, w, tq]
    feeds exp directly in the layout the P@V matmuls consume - no PE
    transposes, no separate P normalize. Normalization is applied after
    P@V: xn = xt * recT where recT is a K=2 parity-broadcast matmul of the
    reciprocal denominators.
  - Per-pair work is software-pipelined (stage A = S+exp for pair p+1
    issued before stage B = denominators/PV/out-proj of pair p) so the PE
    queue never stalls on ACT/DVE round trips.
  - PSUM->SBUF copy work is split across ACT (k zero-pad copies, masked v
    copies, exp) and DVE (q copies, reciprocal, normalize, out copies).

All matmul operands sit at partition base 0 with K in {2, 128} or K=64 at
base 0: mixing row-group offsets across back-to-back K<128 matmuls crashes
the PE.

Biases are zero in this problem; nonzero bv/bp fold into a host-side
output add, nonzero bq/bk fall back to a host computation.
"""

import sys

for _p in ("/opt/trn_rl_repo",):
    if _p not in sys.path:
        sys.path.append(_p)

import numpy as np
import ml_dtypes

import concourse.bacc as bacc
import concourse.bass as bass
import concourse.mybir as mybir
import concourse.tile as tile
from concourse.bass import ds, ts
from concourse.bass_utils import run_bass_kernel_spmd

BF16 = mybir.dt.bfloat16
F32 = mybir.dt.float32
FP8 = mybir.dt.float8e4
NP_BF16 = ml_dtypes.bfloat16
NP_FP8 = ml_dtypes.float8_e4m3  # TRN FP8_EXP4: max +-240, inf beyond

NCORES = 8
DIM = 512
HEADS = 8
HD = 64
N = 64  # window length
B_TOTAL = 2048
WIN_PER_CORE = B_TOTAL // NCORES  # 256
TOK_PER_CORE = WIN_PER_CORE * N  # 16384
SCALE = HD ** -0.5
TT = 512  # token tile (8 windows, 4 pairs)
PAIRS = TT // 128  # 4
# fp8 weight pre-scales (keep e4m3 values out of the subnormal range);
# folded back out in the PSUM->SBUF copy after each projection.
SQ = 128.0
SK = 16.0


def build_program(win_per_core=WIN_PER_CORE):
    """Build the per-core Tile program. All 8 cores run it SPMD."""
    tok = win_per_core * N
    n_tt = tok // TT

    nc = bacc.Bacc("TRN2", debug=False)

    qT_d = nc.dram_tensor("qT", [n_tt, 128, 4, TT], FP8, kind="ExternalInput")
    kT_d = nc.dram_tensor("kT", [n_tt, 128, 4, TT], FP8, kind="ExternalInput")
    vT_d = nc.dram_tensor("vT", [n_tt, 128, 4, TT], BF16, kind="ExternalInput")
    # msel[p, pr, w, j] = mask01 of window (pr,w) token p%64 if p//64==j else 0
    msel_d = nc.dram_tensor(
        "msel", [n_tt, 128, PAIRS, 2, 2], BF16, kind="ExternalInput"
    )
    # m01[p, ct] = mask01 of T-tile token ct*128+p (v-row masking)
    m01_d = nc.dram_tensor("m01", [n_tt, 128, PAIRS], F32, kind="ExternalInput")
    # fp8 DoubleRow weights: [ki 128, kpair 2, ko 2, fo 512], input feature
    # f = (2*kp + ko)*128 + ki
    wq = nc.dram_tensor("wq", [128, 2, 2, DIM], FP8, kind="ExternalInput")
    wk = nc.dram_tensor("wk", [128, 2, 2, DIM], FP8, kind="ExternalInput")
    wv = nc.dram_tensor("wv", [DIM, DIM], BF16, kind="ExternalInput")
    wp = nc.dram_tensor("wp", [DIM, DIM], BF16, kind="ExternalInput")
    sel01 = nc.dram_tensor("sel01", [2, 128], BF16, kind="ExternalInput")
    out = nc.dram_tensor("out", [tok, DIM], BF16, kind="ExternalOutput")

    qa, ka, va = qT_d.ap(), kT_d.ap(), vT_d.ap()
    oa = out.ap()

    with tile.TileContext(nc) as tc:
        with (
            tc.tile_pool(name="consts", bufs=1) as consts,
            tc.tile_pool(name="pin", bufs=3) as pin,
            tc.tile_pool(name="py", bufs=2) as py,
            tc.tile_pool(name="pp", bufs=3) as pp,
            tc.tile_pool(name="pst", bufs=2) as pst,
            tc.tile_pool(name="pout", bufs=4) as pout,
            tc.tile_pool(name="ps_pj", bufs=2, space="PSUM") as ps_pj,
            tc.tile_pool(name="ps_s", bufs=2, space="PSUM") as ps_s,
            tc.tile_pool(name="ps_dn", bufs=1, space="PSUM") as ps_dn,
            tc.tile_pool(name="ps_rt", bufs=1, space="PSUM") as ps_rt,
            tc.tile_pool(name="ps_x", bufs=1, space="PSUM") as ps_x,
            tc.tile_pool(name="ps_o", bufs=1, space="PSUM") as ps_o,
        ):
            # Weights: fp8 DoubleRow layout for q/k, bf16 [fi_part 128,
            # fi_chunk 4, fo 512] for v/p.
            w_tiles = {}
            for name, t in (("wq", wq), ("wk", wk)):
                wt = consts.tile([128, 2, 2, DIM], FP8, tag=f"w_{name}")
                nc.sync.dma_start(out=wt, in_=t.ap())
                w_tiles[name] = wt
            for name, t in (("wv", wv), ("wp", wp)):
                wt = consts.tile([128, 4, DIM], BF16, tag=f"w_{name}")
                nc.sync.dma_start(
                    out=wt, in_=t.ap().rearrange("(c p) f -> p c f", p=128)
                )
                w_tiles[name] = wt
            sel_t = consts.tile([2, 128], BF16, tag="sel01")
            nc.sync.dma_start(out=sel_t, in_=sel01.ap())

            for tt_i in range(n_tt):
                t0 = tt_i * TT
                # ---- transposed input loads (host pre-transposed) ----
                qT = pin.tile([128, 4, TT], FP8, tag="qT")
                kT = pin.tile([128, 4, TT], FP8, tag="kT")
                vT = pin.tile([128, 4, TT], BF16, tag="vT")
                nc.sync.dma_start(out=qT, in_=qa[tt_i])
                nc.sync.dma_start(out=kT, in_=ka[tt_i])
                nc.sync.dma_start(out=vT, in_=va[tt_i])
                msel_t = pin.tile([128, PAIRS, 2, 2], BF16, tag="msel")
                nc.sync.dma_start(out=msel_t, in_=msel_d.ap()[tt_i])
                m01_t = pin.tile([128, PAIRS], F32, tag="m01")
                nc.sync.dma_start(out=m01_t, in_=m01_d.ap()[tt_i])

                # ---- q projection (fp8 DoubleRow, K=256 per MM) ----
                # -> feature-major bf16 [fo 128, c, tok]; copies on DVE,
                # 1/SQ undoes the fp8 weight pre-scale.
                yqT = py.tile([128, 4, TT], BF16, tag="yqT")
                for co in range(4):
                    ps = ps_pj.tile([128, TT], F32, tag="pj")
                    for kp in range(2):
                        nc.tensor.matmul(
                            ps,
                            lhsT=w_tiles["wq"][:, kp, :, ts(co, 128)],
                            rhs=qT[:, ds(2 * kp, 2), :],
                            start=(kp == 0),
                            stop=(kp == 1),
                            perf_mode=mybir.MatmulPerfMode.DoubleRow,
                        )
                    nc.vector.tensor_scalar_mul(yqT[:, co, :], ps, 1.0 / SQ)

                # ---- k projection -> zero-padded per-head-slot layout ----
                # ykTz[p<64, c, pr, w, 0, tk] = head-2c rows, ykTz[p>=64, c,
                # pr, w, 1, tk] = head-2c+1 rows, other slots zero. The
                # (slot, tk) tail is contiguous, so ykTz[:, c, pr, w] is the
                # block-diag stationary operand of an S.T matmul. Zero slots
                # are never overwritten, so memset only on each pool
                # buffer's first use.
                ykTz = py.tile([128, 4, PAIRS, 2, 2, N], BF16, tag="ykTz")
                if tt_i < 2:
                    nc.gpsimd.memset(ykTz[0:64, :, :, :, 1, :], 0.0)
                    nc.gpsimd.memset(ykTz[64:128, :, :, :, 0, :], 0.0)
                for co in range(4):
                    ps = ps_pj.tile([128, TT], F32, tag="pj")
                    for kp in range(2):
                        nc.tensor.matmul(
                            ps,
                            lhsT=w_tiles["wk"][:, kp, :, ts(co, 128)],
                            rhs=kT[:, ds(2 * kp, 2), :],
                            start=(kp == 0),
                            stop=(kp == 1),
                            perf_mode=mybir.MatmulPerfMode.DoubleRow,
                        )
                    psw = ps.rearrange("p (r w t) -> p r w t", r=PAIRS, w=2)
                    nc.scalar.mul(
                        out=ykTz[0:64, co, :, :, 0, :],
                        in_=psw[0:64],
                        mul=1.0 / SK,
                    )
                    nc.scalar.mul(
                        out=ykTz[64:128, co, :, :, 1, :],
                        in_=psw[64:128],
                        mul=1.0 / SK,
                    )

                # ---- v projection -> masked token-major bf16 ----
                # vh rows of masked tokens are zeroed via the per-partition
                # copy scale (numerator side of the masked softmax).
                yv = py.tile([128, PAIRS, DIM], BF16, tag="yv")
                for ct in range(PAIRS):
                    ps = ps_pj.tile([128, DIM], F32, tag="pj")
                    for ci in range(4):
                        nc.tensor.matmul(
                            ps,
                            lhsT=vT[:, ci, ts(ct, 128)],
                            rhs=w_tiles["wv"][:, ci, :],
                            start=(ci == 0),
                            stop=(ci == 3),
                        )
                    nc.scalar.activation(
                        out=yv[:, ct, :],
                        in_=ps,
                        func=mybir.ActivationFunctionType.Copy,
                        scale=m01_t[:, ds(ct, 1)],
                    )
                # second window's vh rows relocated to partition base 0
                yv_hi = py.tile([64, PAIRS, DIM], BF16, tag="yv_hi")
                nc.gpsimd.dma_start(out=yv_hi, in_=yv[64:128, :, :])

                # ---- attention, software-pipelined over 128-token pairs ----
                def stage_a(pr):
                    """S.T for all 8 heads + exp + odd-parity relocation."""
                    base = pr * 128
                    spT = ps_s.tile([128, 4, 2, N], F32, tag="spT")
                    for ch in range(4):
                        for w in range(2):
                            nc.tensor.matmul(
                                spT[:, ch, w, :],
                                lhsT=ykTz[:, ch, pr, w, :, :],
                                rhs=yqT[:, ch, ds(base + 64 * w, 64)],
                                start=True,
                                stop=True,
                            )
                    pexpT = pp.tile([128, 4, 2, N], BF16, tag="pexpT")
                    nc.scalar.activation(
                        out=pexpT, in_=spT, func=mybir.ActivationFunctionType.Exp
                    )
                    return (pexpT,)

                def stage_b1(pr, pexpT):
                    """Denominators + normalized P.T (pnT)."""
                    # dn[j, ch, w, tq] = sum over parity-j partitions of
                    # msel * pexpT = per-head masked softmax denominators
                    dn = ps_dn.tile([2, 4, 2, N], F32, tag="dn")
                    for w in range(2):
                        nc.tensor.matmul(
                            dn[:, :, w, :],
                            lhsT=msel_t[:, pr, w, :],
                            rhs=pexpT[:, :, w, :],
                            start=True,
                            stop=True,
                        )
                    rec32 = pst.tile([2, 4, 2, N], F32, tag="rec32")
                    nc.vector.reciprocal_approx_fast(
                        out=rec32.rearrange("p a w t -> p (a w t)"),
                        in_=dn.rearrange("p a w t -> p (a w t)"),
                    )
                    rec = pst.tile([2, 4, 2, N], BF16, tag="rec")
                    nc.scalar.mul(out=rec, in_=rec32, mul=1.0)
                    # broadcast rec to the 64-partition parity blocks
                    rT = ps_rt.tile([128, 4, 2, N], F32, tag="rT")
                    nc.tensor.matmul(rT, lhsT=sel_t, rhs=rec, start=True, stop=True)
                    pnT = pp.tile([128, 4, 2, N], BF16, tag="pnT")
                    nc.vector.tensor_tensor(
                        out=pnT, in0=pexpT, in1=rT, op=mybir.AluOpType.mult
                    )
                    pnT_od = pp.tile([64, 4, 2, N], BF16, tag="pnT_od")
                    nc.gpsimd.dma_start(out=pnT_od, in_=pnT[64:128, :, :, :])
                    return pnT, pnT_od

                def stage_b2(pr, pnT, pnT_od):
                    """P@V and output projection."""
                    base = pr * 128
                    # X.T: [feat-in-chunk 128, ch, w, tq]
                    xt = ps_x.tile([128, 4, 2, N], F32, tag="xt")
                    for h in range(HEADS):
                        ch, hh = h // 2, (h % 2) * 64
                        psrc = pnT if h % 2 == 0 else pnT_od
                        for w in range(2):
                            vsrc = yv if w == 0 else yv_hi
                            nc.tensor.matmul(
                                xt[ds(hh, 64), ch, w, :],
                                lhsT=vsrc[ds(0, 64), pr, ts(h, 64)],
                                rhs=psrc[ds(0, 64), ch, w, :],
                                start=True,
                                stop=True,
                            )
                    xts = pp.tile([128, 4, 2, N], BF16, tag="xts")
                    nc.vector.tensor_copy(out=xts, in_=xt)
                    # output projection: [tok 128, fo 512]
                    po = ps_o.tile([128, DIM], F32, tag="po")
                    for c in range(4):
                        nc.tensor.matmul(
                            po,
                            lhsT=xts[:, c, :, :],
                            rhs=w_tiles["wp"][:, c, :],
                            start=(c == 0),
                            stop=(c == 3),
                        )
                    osb = pout.tile([128, DIM], BF16, tag="osb")
                    nc.vector.tensor_copy(out=osb, in_=po)
                    nc.sync.dma_start(
                        out=oa[t0 + base : t0 + base + 128, :], in_=osb
                    )

                pa = [None] * PAIRS
                pb = [None] * PAIRS
                pa[0] = stage_a(0)
                pa[1] = stage_a(1)
                pb[0] = stage_b1(0, *pa[0])
                pa[2] = stage_a(2)
                stage_b2(0, *pb[0])
                pb[1] = stage_b1(1, *pa[1])
                pa[3] = stage_a(3)
                stage_b2(1, *pb[1])
                pb[2] = stage_b1(2, *pa[2])
                stage_b2(2, *pb[2])
                pb[3] = stage_b1(3, *pa[3])
                stage_b2(3, *pb[3])

    nc.compile()
    return nc


_PROGRAM_CACHE = {}


def _get_program(win_per_core):
    if win_per_core not in _PROGRAM_CACHE:
        _PROGRAM_CACHE[win_per_core] = build_program(win_per_core)
    return _PROGRAM_CACHE[win_per_core]


def _feature_major_tiles(x_flat):
    """[tok, 512] -> [n_tt, 128, 4, TT] so each T-tile block is one
    fully-contiguous transposed DMA read."""
    tok = x_flat.shape[0]
    n_tt = tok // TT
    xt = x_flat.reshape(n_tt, TT, 4, 128).transpose(0, 3, 2, 1)
    return np.ascontiguousarray(xt)


def _fp8(x):
    return np.clip(np.asarray(x, np.float32), -240.0, 240.0).astype(NP_FP8)


def _dr_weights(wT_scaled):
    """[512 fi, 512 fo] -> DoubleRow fp8 layout [ki 128, kp 2, ko 2, fo]."""
    w8 = _fp8(wT_scaled)
    return np.ascontiguousarray(
        w8.reshape(2, 2, 128, DIM).transpose(2, 0, 1, 3)
    )


def make_in_maps(q, k, v, mask, Wq, Wk, Wv, Wp, ncores=NCORES):
    """Host-side shard + layout prep. Returns list of per-core input dicts."""
    B, n, C = q.shape
    win_pc = B // ncores
    qf = np.ascontiguousarray(q.reshape(B * n, C))
    kf = np.ascontiguousarray(k.reshape(B * n, C))
    vf = np.ascontiguousarray(v.reshape(B * n, C))

    mflat = (mask != 0).astype(np.float32).reshape(B * n)

    wq_t = _dr_weights(Wq.T * (SCALE * SQ))
    wk_t = _dr_weights(Wk.T * SK)
    wv_t = np.ascontiguousarray(Wv.T.astype(NP_BF16))
    wp_t = np.ascontiguousarray(Wp.T.astype(NP_BF16))
    sel = np.zeros((2, 128), NP_BF16)
    sel[0, 0:64] = 1
    sel[1, 64:128] = 1

    tok_pc = win_pc * n
    n_tt = tok_pc // TT
    in_maps = []
    for c in range(ncores):
        sl = slice(c * tok_pc, (c + 1) * tok_pc)
        mc = mflat[sl]
        # m01[tt, p, ct] = mask of token tt*512 + ct*128 + p
        m01 = np.ascontiguousarray(
            mc.reshape(n_tt, PAIRS, 128).transpose(0, 2, 1)
        )
        # msel[tt, p, pr, w, j]: parity-j selector rows carry the window
        # (pr, w) mask at tk = p % 64
        mwin = mc.reshape(n_tt, PAIRS, 2, 64).transpose(0, 3, 1, 2)
        msel = np.zeros((n_tt, 128, PAIRS, 2, 2), NP_BF16)
        msel[:, 0:64, :, :, 0] = mwin
        msel[:, 64:128, :, :, 1] = mwin
        in_maps.append(
            {
                "qT": _feature_major_tiles(_fp8(qf[sl])),
                "kT": _feature_major_tiles(_fp8(kf[sl])),
                "vT": _feature_major_tiles(vf[sl].astype(NP_BF16)),
                "msel": msel,
                "m01": m01,
                "wq": wq_t,
                "wk": wk_t,
                "wv": wv_t,
                "wp": wp_t,
                "sel01": sel,
            }
        )
    return in_maps


def _reference_numpy(q, k, v, mask, Wq, bq, Wk, bk, Wv, bv, Wp, bp):
    """Full-precision host fallback (only used for nonzero bq/bk)."""
    B, n, C = q.shape
    qh = (q.reshape(-1, C) @ Wq.T + bq).reshape(B, n, HEADS, HD).transpose(0, 2, 1, 3)
    kh = (k.reshape(-1, C) @ Wk.T + bk).reshape(B, n, HEADS, HD).transpose(0, 2, 1, 3)
    vh = (v.reshape(-1, C) @ Wv.T + bv).reshape(B, n, HEADS, HD).transpose(0, 2, 1, 3)
    s = np.einsum("bhqd,bhkd->bhqk", qh * SCALE, kh)
    s = np.where((mask[:, None, None, :] == 0), np.float32(-10000.0), s)
    s = s - s.max(-1, keepdims=True)
    e = np.exp(s)
    p = e / e.sum(-1, keepdims=True)
    x = np.einsum("bhqk,bhkd->bhqd", p, vh)
    x = x.transpose(0, 2, 1, 3).reshape(B, n, C)
    return (x @ Wp.T + bp).astype(np.float32)


def kernel(q, k, v, mask, Wq, bq, Wk, bk, Wv, bv, Wp, bp, trace=False):
    q = np.asarray(q, np.float32)
    k = np.asarray(k, np.float32)
    v = np.asarray(v, np.float32)
    mask = np.asarray(mask)
    Wq, Wk, Wv, Wp = (np.asarray(w, np.float32) for w in (Wq, Wk, Wv, Wp))
    bq, bk, bv, bp = (np.asarray(b, np.float32) for b in (bq, bk, bv, bp))

    if np.any(bq) or np.any(bk):
        return _reference_numpy(q, k, v, mask, Wq, bq, Wk, bk, Wv, bv, Wp, bp)

    B, n, C = q.shape
    win_pc = B // NCORES
    nc = _get_program(win_pc)
    in_maps = make_in_maps(q, k, v, mask, Wq, Wk, Wv, Wp)
    res = run_bass_kernel_spmd(
        nc, in_maps, core_ids=list(range(NCORES)), trace=trace
    )
    outs = np.concatenate(
        [np.asarray(r["out"], np.float32) for r in res.results], axis=0
    )
    outs = outs.reshape(B, n, C)
    # bv flows through attention linearly (softmax rows sum to 1); with bp it
    # folds into a single output bias.
    bout = bp + bv @ Wp.T
    if np.any(bout):
        outs = outs + bout.astype(np.float32)
    if trace:
        kernel._last_result = res
    return outs


# revision 36
# speedup vs baseline: 1.4231x; 1.1014x over previous
"""Bass/Trainium2 kernel for windowed multi-head attention (8 NeuronCores).

Math per window b (64 tokens), matching torch-style nn.MultiHeadAttention:
  qh = (q @ Wq.T + bq) * scale; kh = k @ Wk.T + bk; vh = v @ Wv.T + bv
  S  = qh @ kh.T  (per head);  S[masked k] = -1e4;  P = softmax(S)
  out = concat_h(P @ vh) @ Wp.T + bp

Device dataflow (per core, 256 windows = 16384 tokens; f32 PSUM accum):
  - q,k are fed as fp8(e4m3) feature-major [128, 4, TT] T-tile blocks and
    projected with fp8 DoubleRow matmuls (K=256 per MM); the fp8 weight
    pre-scales (SQ/SK, needed to clear e4m3's subnormal cutoff) are folded
    back out in the PSUM->SBUF copies. v stays bf16.
  - The mask never touches S. Instead masked softmax is computed as
       x = exp(S) @ (m * vh) / (exp(S) @ m)
    i.e. vh rows are zeroed during the v PSUM->SBUF copy (per-partition
    scale), and per-head denominators come from a K=128 matmul of exp(S).T
    against a masked parity selector.
  - S is computed TRANSPOSED from the start: lhsT = zero-padded per-head-
    slot k layout (block-diag over the 2 heads sharing a 128-feature
    chunk), rhs = feature-major qh. The result spT[(parity,tk), ch, w, tq]
    feeds exp directly in the layout the P@V matmuls consume - no PE
    transposes, no separate P normalize. Normalization is applied after
    P@V: xn = xt * recT where recT is a K=2 parity-broadcast matmul of the
    reciprocal denominators.
  - Per-pair work is software-pipelined (stage A = S+exp for pair p+1
    issued before stage B = denominators/PV/out-proj of pair p) so the PE
    queue never stalls on ACT/DVE round trips.
  - PSUM->SBUF copy work is split across ACT (k zero-pad copies, masked v
    copies, exp) and DVE (q copies, reciprocal, normalize, out copies).

All matmul operands sit at partition base 0 with K in {2, 128} or K=64 at
base 0: mixing row-group offsets across back-to-back K<128 matmuls crashes
the PE.

Biases are zero in this problem; nonzero bv/bp fold into a host-side
output add, nonzero bq/bk fall back to a host computation.
"""

import sys

for _p in ("/opt/trn_rl_repo",):
    if _p not in sys.path:
        sys.path.append(_p)

import numpy as np
import ml_dtypes

import concourse.bacc as bacc
import concourse.bass as bass
import concourse.mybir as mybir
import concourse.tile as tile
from concourse.bass import ds, ts
from concourse.bass_utils import run_bass_kernel_spmd

BF16 = mybir.dt.bfloat16
F32 = mybir.dt.float32
FP8 = mybir.dt.float8e4
NP_BF16 = ml_dtypes.bfloat16
NP_FP8 = ml_dtypes.float8_e4m3  # TRN FP8_EXP4: max +-240, inf beyond

NCORES = 8
DIM = 512
HEADS = 8
HD = 64
N = 64  # window length
B_TOTAL = 2048
WIN_PER_CORE = B_TOTAL // NCORES  # 256
TOK_PER_CORE = WIN_PER_CORE * N  # 16384
SCALE = HD ** -0.5
TT = 512  # token tile (8 windows, 4 pairs)
PAIRS = TT // 128  # 4
# fp8 weight pre-scales (keep e4m3 values out of the subnormal range);
# folded back out in the PSUM->SBUF copy after each projection.
SQ = 128.0
SK = 16.0


def build_program(win_per_core=WIN_PER_CORE):
    """Build the per-core Tile program. All 8 cores run it SPMD."""
    tok = win_per_core * N
    n_tt = tok // TT

    nc = bacc.Bacc("TRN2", debug=False)

    qT_d = nc.dram_tensor("qT", [n_tt, 128, 4, TT], FP8, kind="ExternalInput")
    kT_d = nc.dram_tensor("kT", [n_tt, 128, 4, TT], FP8, kind="ExternalInput")
    vT_d = nc.dram_tensor("vT", [n_tt, 128, 4, TT], BF16, kind="ExternalInput")
    # msel[p, pr, w, m] = mask01 of window (pr,w) token p%64 if p//64==m//64
    # else 0: M=128 duplicated columns so the denominator matmul directly
    # produces per-partition-broadcast denominators.
    msel_d = nc.dram_tensor(
        "msel", [n_tt, 128, PAIRS, 2, 128], BF16, kind="ExternalInput"
    )
    # m01[p, ct] = mask01 of T-tile token ct*128+p (v-row masking)
    m01_d = nc.dram_tensor("m01", [n_tt, 128, PAIRS], F32, kind="ExternalInput")
    # fp8 DoubleRow weights: [ki 128, kpair 2, ko 2, fo 512], input feature
    # f = (2*kp + ko)*128 + ki
    wq = nc.dram_tensor("wq", [128, 2, 2, DIM], FP8, kind="ExternalInput")
    wk = nc.dram_tensor("wk", [128, 2, 2, DIM], FP8, kind="ExternalInput")
    wv = nc.dram_tensor("wv", [DIM, DIM], BF16, kind="ExternalInput")
    wp = nc.dram_tensor("wp", [DIM, DIM], BF16, kind="ExternalInput")
    out = nc.dram_tensor("out", [tok, DIM], BF16, kind="ExternalOutput")

    qa, ka, va = qT_d.ap(), kT_d.ap(), vT_d.ap()
    oa = out.ap()

    with tile.TileContext(nc) as tc:
        with (
            tc.tile_pool(name="consts", bufs=1) as consts,
            tc.tile_pool(name="pin", bufs=3) as pin,
            tc.tile_pool(name="py", bufs=2) as py,
            tc.tile_pool(name="pp", bufs=3) as pp,
            tc.tile_pool(name="pst", bufs=2) as pst,
            tc.tile_pool(name="pout", bufs=4) as pout,
            tc.tile_pool(name="ps_pj", bufs=2, space="PSUM") as ps_pj,
            tc.tile_pool(name="ps_s", bufs=2, space="PSUM") as ps_s,
            tc.tile_pool(name="ps_dn", bufs=1, space="PSUM") as ps_dn,
            tc.tile_pool(name="ps_x", bufs=1, space="PSUM") as ps_x,
            tc.tile_pool(name="ps_o", bufs=2, space="PSUM") as ps_o,
        ):
            # Weights: fp8 DoubleRow layout for q/k, bf16 [fi_part 128,
            # fi_chunk 4, fo 512] for v/p.
            w_tiles = {}
            for name, t in (("wq", wq), ("wk", wk)):
                wt = consts.tile([128, 2, 2, DIM], FP8, tag=f"w_{name}")
                nc.sync.dma_start(out=wt, in_=t.ap())
                w_tiles[name] = wt
            for name, t in (("wv", wv), ("wp", wp)):
                wt = consts.tile([128, 4, DIM], BF16, tag=f"w_{name}")
                nc.sync.dma_start(
                    out=wt, in_=t.ap().rearrange("(c p) f -> p c f", p=128)
                )
                w_tiles[name] = wt

            for tt_i in range(n_tt):
                t0 = tt_i * TT
                # ---- transposed input loads (host pre-transposed) ----
                qT = pin.tile([128, 4, TT], FP8, tag="qT")
                kT = pin.tile([128, 4, TT], FP8, tag="kT")
                vT = pin.tile([128, 4, TT], BF16, tag="vT")
                nc.sync.dma_start(out=qT, in_=qa[tt_i])
                nc.sync.dma_start(out=kT, in_=ka[tt_i])
                nc.sync.dma_start(out=vT, in_=va[tt_i])
                msel_t = pin.tile([128, PAIRS, 2, 128], BF16, tag="msel")
                nc.sync.dma_start(out=msel_t, in_=msel_d.ap()[tt_i])
                m01_t = pin.tile([128, PAIRS], F32, tag="m01")
                nc.sync.dma_start(out=m01_t, in_=m01_d.ap()[tt_i])

                # ---- v projection -> masked token-major bf16 ----
                # (first so its ACT copies don't delay exp; vh rows of
                # masked tokens are zeroed via the per-partition copy scale
                # = numerator side of the masked softmax)
                yv = py.tile([128, PAIRS, DIM], BF16, tag="yv")
                for ct in range(PAIRS):
                    ps = ps_pj.tile([128, DIM], F32, tag="pj")
                    for ci in range(4):
                        nc.tensor.matmul(
                            ps,
                            lhsT=vT[:, ci, ts(ct, 128)],
                            rhs=w_tiles["wv"][:, ci, :],
                            start=(ci == 0),
                            stop=(ci == 3),
                        )
                    nc.scalar.activation(
                        out=yv[:, ct, :],
                        in_=ps,
                        func=mybir.ActivationFunctionType.Copy,
                        scale=m01_t[:, ds(ct, 1)],
                    )
                # second window's vh rows relocated to partition base 0
                yv_hi = py.tile([64, PAIRS, DIM], BF16, tag="yv_hi")
                nc.gpsimd.dma_start(out=yv_hi, in_=yv[64:128, :, :])

                # ---- k projection -> zero-padded per-head-slot layout ----
                # ykTz[p<64, c, pr, w, 0, tk] = head-2c rows, ykTz[p>=64, c,
                # pr, w, 1, tk] = head-2c+1 rows, other slots zero. The
                # (slot, tk) tail is contiguous, so ykTz[:, c, pr, w] is the
                # block-diag stationary operand of an S.T matmul. Zero slots
                # are never overwritten, so memset only on each pool
                # buffer's first use.
                ykTz = py.tile([128, 4, PAIRS, 2, 2, N], BF16, tag="ykTz")
                if tt_i < 2:
                    nc.gpsimd.memset(ykTz[0:64, :, :, :, 1, :], 0.0)
                    nc.gpsimd.memset(ykTz[64:128, :, :, :, 0, :], 0.0)
                for co in range(4):
                    ps = ps_pj.tile([128, TT], F32, tag="pj")
                    for kp in range(2):
                        nc.tensor.matmul(
                            ps,
                            lhsT=w_tiles["wk"][:, kp, :, ts(co, 128)],
                            rhs=kT[:, ds(2 * kp, 2), :],
                            start=(kp == 0),
                            stop=(kp == 1),
                            perf_mode=mybir.MatmulPerfMode.DoubleRow,
                        )
                    psw = ps.rearrange("p (r w t) -> p r w t", r=PAIRS, w=2)
                    nc.scalar.mul(
                        out=ykTz[0:64, co, :, :, 0, :],
                        in_=psw[0:64],
                        mul=1.0 / SK,
                    )
                    nc.scalar.mul(
                        out=ykTz[64:128, co, :, :, 1, :],
                        in_=psw[64:128],
                        mul=1.0 / SK,
                    )

                # ---- q projection (fp8 DoubleRow, K=256 per MM) ----
                # -> feature-major bf16 [fo 128, c, tok]; copies on DVE,
                # 1/SQ undoes the fp8 weight pre-scale.
                yqT = py.tile([128, 4, TT], BF16, tag="yqT")
                for co in range(4):
                    ps = ps_pj.tile([128, TT], F32, tag="pj")
                    for kp in range(2):
                        nc.tensor.matmul(
                            ps,
                            lhsT=w_tiles["wq"][:, kp, :, ts(co, 128)],
                            rhs=qT[:, ds(2 * kp, 2), :],
                            start=(kp == 0),
                            stop=(kp == 1),
                            perf_mode=mybir.MatmulPerfMode.DoubleRow,
                        )
                    nc.vector.tensor_scalar_mul(yqT[:, co, :], ps, 1.0 / SQ)

                # ---- attention, software-pipelined over 128-token pairs ----
                def stage_a(pr):
                    """S.T for all 8 heads + exp + odd-parity relocation."""
                    base = pr * 128
                    spT = ps_s.tile([128, 4, 2, N], F32, tag="spT")
                    for ch in range(4):
                        for w in range(2):
                            nc.tensor.matmul(
                                spT[:, ch, w, :],
                                lhsT=ykTz[:, ch, pr, w, :, :],
                                rhs=yqT[:, ch, ds(base + 64 * w, 64)],
                                start=True,
                                stop=True,
                            )
                    pexpT = pp.tile([128, 4, 2, N], BF16, tag="pexpT")
                    nc.scalar.activation(
                        out=pexpT, in_=spT, func=mybir.ActivationFunctionType.Exp
                    )
                    pexpT_od = pp.tile([64, 4, 2, N], BF16, tag="pexpT_od")
                    nc.gpsimd.dma_start(out=pexpT_od, in_=pexpT[64:128, :, :, :])
                    return pexpT, pexpT_od

                def stage_b1(pr, pexpT, pexpT_od):
                    """Masked softmax denominators -> broadcast reciprocals.

                    Runs in parallel with the (unnormalized) P@V matmuls;
                    normalization is linear so it is applied to X instead
                    of P. The M=128 duplicated-column msel makes the
                    denominators land already broadcast per parity block."""
                    dn = ps_dn.tile([128, 4, 2, N], F32, tag="dn")
                    for w in range(2):
                        nc.tensor.matmul(
                            dn[:, :, w, :],
                            lhsT=msel_t[:, pr, w, :],
                            rhs=pexpT[:, :, w, :],
                            start=True,
                            stop=True,
                        )
                    recb = pst.tile([128, 4, 2, N], F32, tag="recb")
                    nc.vector.reciprocal_approx_fast(
                        out=recb.rearrange("p a w t -> p (a w t)"),
                        in_=dn.rearrange("p a w t -> p (a w t)"),
                    )
                    return recb

                def stage_b2x(pr, pexpT, pexpT_od, recb):
                    """Unnormalized P@V, then normalize X by recb."""
                    xt = ps_x.tile([128, 4, 2, N], F32, tag="xt")
                    for h in range(HEADS):
                        ch, hh = h // 2, (h % 2) * 64
                        psrc = pexpT if h % 2 == 0 else pexpT_od
                        for w in range(2):
                            vsrc = yv if w == 0 else yv_hi
                            nc.tensor.matmul(
                                xt[ds(hh, 64), ch, w, :],
                                lhsT=vsrc[ds(0, 64), pr, ts(h, 64)],
                                rhs=psrc[ds(0, 64), ch, w, :],
                                start=True,
                                stop=True,
                            )
                    xts = pp.tile([128, 4, 2, N], BF16, tag="xts")
                    nc.vector.tensor_tensor(
                        out=xts, in0=xt, in1=recb, op=mybir.AluOpType.mult
                    )
                    return xts

                def stage_b2o(pr, xts):
                    """Output projection: [tok 128, fo 512]."""
                    base = pr * 128
                    po = ps_o.tile([128, DIM], F32, tag="po")
                    for c in range(4):
                        nc.tensor.matmul(
                            po,
                            lhsT=xts[:, c, :, :],
                            rhs=w_tiles["wp"][:, c, :],
                            start=(c == 0),
                            stop=(c == 3),
                        )
                    osb = pout.tile([128, DIM], BF16, tag="osb")
                    nc.vector.tensor_copy(out=osb, in_=po)
                    nc.sync.dma_start(
                        out=oa[t0 + base : t0 + base + 128, :], in_=osb
                    )

                pa = [None] * PAIRS
                prec = [None] * PAIRS
                pxts = [None] * PAIRS
                pa[0] = stage_a(0)
                pa[1] = stage_a(1)
                prec[0] = stage_b1(0, *pa[0])
                pa[2] = stage_a(2)
                prec[1] = stage_b1(1, *pa[1])
                pxts[0] = stage_b2x(0, *pa[0], prec[0])
                pa[3] = stage_a(3)
                prec[2] = stage_b1(2, *pa[2])
                pxts[1] = stage_b2x(1, *pa[1], prec[1])
                stage_b2o(0, pxts[0])
                prec[3] = stage_b1(3, *pa[3])
                pxts[2] = stage_b2x(2, *pa[2], prec[2])
                stage_b2o(1, pxts[1])
                pxts[3] = stage_b2x(3, *pa[3], prec[3])
                stage_b2o(2, pxts[2])
                stage_b2o(3, pxts[3])

    nc.compile()
    return nc


_PROGRAM_CACHE = {}


def _get_program(win_per_core):
    if win_per_core not in _PROGRAM_CACHE:
        _PROGRAM_CACHE[win_per_core] = build_program(win_per_core)
    return _PROGRAM_CACHE[win_per_core]


def _feature_major_tiles(x_flat):
    """[tok, 512] -> [n_tt, 128, 4, TT] so each T-tile block is one
    fully-contiguous transposed DMA read."""
    tok = x_flat.shape[0]
    n_tt = tok // TT
    xt = x_flat.reshape(n_tt, TT, 4, 128).transpose(0, 3, 2, 1)
    return np.ascontiguousarray(xt)


def _fp8(x):
    return np.clip(np.asarray(x, np.float32), -240.0, 240.0).astype(NP_FP8)


def _dr_weights(wT_scaled):
    """[512 fi, 512 fo] -> DoubleRow fp8 layout [ki 128, kp 2, ko 2, fo]."""
    w8 = _fp8(wT_scaled)
    return np.ascontiguousarray(
        w8.reshape(2, 2, 128, DIM).transpose(2, 0, 1, 3)
    )


def make_in_maps(q, k, v, mask, Wq, Wk, Wv, Wp, ncores=NCORES):
    """Host-side shard + layout prep. Returns list of per-core input dicts."""
    B, n, C = q.shape
    win_pc = B // ncores
    qf = np.ascontiguousarray(q.reshape(B * n, C))
    kf = np.ascontiguousarray(k.reshape(B * n, C))
    vf = np.ascontiguousarray(v.reshape(B * n, C))

    mflat = (mask != 0).astype(np.float32).reshape(B * n)

    wq_t = _dr_weights(Wq.T * (SCALE * SQ))
    wk_t = _dr_weights(Wk.T * SK)
    wv_t = np.ascontiguousarray(Wv.T.astype(NP_BF16))
    wp_t = np.ascontiguousarray(Wp.T.astype(NP_BF16))

    tok_pc = win_pc * n
    n_tt = tok_pc // TT
    in_maps = []
    for c in range(ncores):
        sl = slice(c * tok_pc, (c + 1) * tok_pc)
        mc = mflat[sl]
        # m01[tt, p, ct] = mask of token tt*512 + ct*128 + p
        m01 = np.ascontiguousarray(
            mc.reshape(n_tt, PAIRS, 128).transpose(0, 2, 1)
        )
        # msel[tt, p, pr, w, m]: parity selector rows carry the window
        # (pr, w) mask at tk = p % 64, duplicated over 64 m-columns per
        # parity so the denominator matmul output is partition-broadcast
        mwin = mc.reshape(n_tt, PAIRS, 2, 64).transpose(0, 3, 1, 2)[..., None]
        msel = np.zeros((n_tt, 128, PAIRS, 2, 128), NP_BF16)
        msel[:, 0:64, :, :, 0:64] = mwin
        msel[:, 64:128, :, :, 64:128] = mwin
        in_maps.append(
            {
                "qT": _feature_major_tiles(_fp8(qf[sl])),
                "kT": _feature_major_tiles(_fp8(kf[sl])),
                "vT": _feature_major_tiles(vf[sl].astype(NP_BF16)),
                "msel": msel,
                "m01": m01,
                "wq": wq_t,
                "wk": wk_t,
                "wv": wv_t,
                "wp": wp_t,
            }
        )
    return in_maps


def _reference_numpy(q, k, v, mask, Wq, bq, Wk, bk, Wv, bv, Wp, bp):
    """Full-precision host fallback (only used for nonzero bq/bk)."""
    B, n, C = q.shape
    qh = (q.reshape(-1, C) @ Wq.T + bq).reshape(B, n, HEADS, HD).transpose(0, 2, 1, 3)
    kh = (k.reshape(-1, C) @ Wk.T + bk).reshape(B, n, HEADS, HD).transpose(0, 2, 1, 3)
    vh = (v.reshape(-1, C) @ Wv.T + bv).reshape(B, n, HEADS, HD).transpose(0, 2, 1, 3)
    s = np.einsum("bhqd,bhkd->bhqk", qh * SCALE, kh)
    s = np.where((mask[:, None, None, :] == 0), np.float32(-10000.0), s)
    s = s - s.max(-1, keepdims=True)
    e = np.exp(s)
    p = e / e.sum(-1, keepdims=True)
    x = np.einsum("bhqk,bhkd->bhqd", p, vh)
    x = x.transpose(0, 2, 1, 3).reshape(B, n, C)
    return (x @ Wp.T + bp).astype(np.float32)


def kernel(q, k, v, mask, Wq, bq, Wk, bk, Wv, bv, Wp, bp, trace=False):
    q = np.asarray(q, np.float32)
    k = np.asarray(k, np.float32)
    v = np.asarray(v, np.float32)
    mask = np.asarray(mask)
    Wq, Wk, Wv, Wp = (np.asarray(w, np.float32) for w in (Wq, Wk, Wv, Wp))
    bq, bk, bv, bp = (np.asarray(b, np.float32) for b in (bq, bk, bv, bp))

    if np.any(bq) or np.any(bk):
        return _reference_numpy(q, k, v, mask, Wq, bq, Wk, bk, Wv, bv, Wp, bp)

    B, n, C = q.shape
    win_pc = B // NCORES
    nc = _get_program(win_pc)
    in_maps = make_in_maps(q, k, v, mask, Wq, Wk, Wv, Wp)
    res = run_bass_kernel_spmd(
        nc, in_maps, core_ids=list(range(NCORES)), trace=trace
    )
    outs = np.concatenate(
        [np.asarray(r["out"], np.float32) for r in res.results], axis=0
    )
    outs = outs.reshape(B, n, C)
    # bv flows through attention linearly (softmax rows sum to 1); with bp it
    # folds into a single output bias.
    bout = bp + bv @ Wp.T
    if np.any(bout):
        outs = outs + bout.astype(np.float32)
    if trace:
        kernel._last_result = res
    return outs


# revision 39
# speedup vs baseline: 1.5732x; 1.1055x over previous
"""Bass/Trainium2 kernel for windowed multi-head attention (8 NeuronCores).

Math per window b (64 tokens), matching torch-style nn.MultiHeadAttention:
  qh = (q @ Wq.T + bq) * scale; kh = k @ Wk.T + bk; vh = v @ Wv.T + bv
  S  = qh @ kh.T  (per head);  S[masked k] = -1e4;  P = softmax(S)
  out = concat_h(P @ vh) @ Wp.T + bp

Device dataflow (per core, 256 windows = 16384 tokens; f32 PSUM accum):
  - q,k are fed as fp8(e4m3) feature-major [128, 4, TT] T-tile blocks and
    projected with fp8 DoubleRow matmuls (K=256 per MM); the fp8 weight
    pre-scales (SQ/SK, needed to clear e4m3's subnormal cutoff) are folded
    back out in the PSUM->SBUF copies. v stays bf16.
  - The mask never touches S. Instead masked softmax is computed as
       x = exp(S) @ (m * vh) / (exp(S) @ m)
    i.e. vh rows are zeroed during the v PSUM->SBUF copy (per-partition
    scale), and per-head denominators come from a K=128 matmul of exp(S).T
    against a masked parity selector.
  - S is computed TRANSPOSED from the start: lhsT = zero-padded per-head-
    slot k layout (block-diag over the 2 heads sharing a 128-feature
    chunk), rhs = feature-major qh. The result spT[(parity,tk), ch, w, tq]
    feeds exp directly in the layout the P@V matmuls consume - no PE
    transposes, no separate P normalize. Normalization is applied after
    P@V: xn = xt * recT where recT is a K=2 parity-broadcast matmul of the
    reciprocal denominators.
  - Per-pair work is software-pipelined (stage A = S+exp for pair p+1
    issued before stage B = denominators/PV/out-proj of pair p) so the PE
    queue never stalls on ACT/DVE round trips.
  - PSUM->SBUF copy work is split across ACT (k zero-pad copies, masked v
    copies, exp) and DVE (q copies, reciprocal, normalize, out copies).

All matmul operands sit at partition base 0 with K in {2, 128} or K=64 at
base 0: mixing row-group offsets across back-to-back K<128 matmuls crashes
the PE.

Biases are zero in this problem; nonzero bv/bp fold into a host-side
output add, nonzero bq/bk fall back to a host computation.
"""

import sys

for _p in ("/opt/trn_rl_repo",):
    if _p not in sys.path:
        sys.path.append(_p)

import numpy as np
import ml_dtypes

import concourse.bacc as bacc
import concourse.bass as bass
import concourse.mybir as mybir
import concourse.tile as tile
from concourse.bass import ds, ts
from concourse.bass_utils import run_bass_kernel_spmd

BF16 = mybir.dt.bfloat16
F32 = mybir.dt.float32
FP8 = mybir.dt.float8e4
NP_BF16 = ml_dtypes.bfloat16
NP_FP8 = ml_dtypes.float8_e4m3  # TRN FP8_EXP4: max +-240, inf beyond

NCORES = 8
DIM = 512
HEADS = 8
HD = 64
N = 64  # window length
B_TOTAL = 2048
WIN_PER_CORE = B_TOTAL // NCORES  # 256
TOK_PER_CORE = WIN_PER_CORE * N  # 16384
SCALE = HD ** -0.5
TT = 512  # token tile (8 windows, 4 pairs)
PAIRS = TT // 128  # 4
# fp8 weight pre-scales (keep e4m3 values out of the subnormal range);
# folded back out in the PSUM->SBUF copy after each projection.
SQ = 128.0
SK = 16.0


def build_program(win_per_core=WIN_PER_CORE):
    """Build the per-core Tile program. All 8 cores run it SPMD."""
    tok = win_per_core * N
    n_tt = tok // TT

    nc = bacc.Bacc("TRN2", debug=False)

    qT_d = nc.dram_tensor("qT", [n_tt, 128, 4, TT], FP8, kind="ExternalInput")
    kT_d = nc.dram_tensor("kT", [n_tt, 128, 4, TT], FP8, kind="ExternalInput")
    vT_d = nc.dram_tensor("vT", [n_tt, 128, 4, TT], BF16, kind="ExternalInput")
    # msel[p, pr, w, m] = mask01 of window (pr,w) token p%64 if p//64==m//64
    # else 0: M=128 duplicated columns so the denominator matmul directly
    # produces per-partition-broadcast denominators.
    msel_d = nc.dram_tensor(
        "msel", [n_tt, 128, PAIRS, 2, 128], BF16, kind="ExternalInput"
    )
    # m01[p, ct] = mask01 of T-tile token ct*128+p (v-row masking)
    m01_d = nc.dram_tensor("m01", [n_tt, 128, PAIRS], F32, kind="ExternalInput")
    # fp8 DoubleRow weights: [ki 128, kpair 2, ko 2, fo 512], input feature
    # f = (2*kp + ko)*128 + ki
    wq = nc.dram_tensor("wq", [128, 2, 2, DIM], FP8, kind="ExternalInput")
    wk = nc.dram_tensor("wk", [128, 2, 2, DIM], FP8, kind="ExternalInput")
    wv = nc.dram_tensor("wv", [DIM, DIM], BF16, kind="ExternalInput")
    wp = nc.dram_tensor("wp", [DIM, DIM], BF16, kind="ExternalInput")
    out = nc.dram_tensor("out", [tok, DIM], BF16, kind="ExternalOutput")

    qa, ka, va = qT_d.ap(), kT_d.ap(), vT_d.ap()
    oa = out.ap()

    with tile.TileContext(nc) as tc:
        with (
            tc.tile_pool(name="consts", bufs=1) as consts,
            tc.tile_pool(name="pin", bufs=3) as pin,
            tc.tile_pool(name="py", bufs=2) as py,
            tc.tile_pool(name="pp", bufs=3) as pp,
            tc.tile_pool(name="pst", bufs=2) as pst,
            tc.tile_pool(name="pout", bufs=4) as pout,
            tc.tile_pool(name="ps_pj", bufs=3, space="PSUM") as ps_pj,
            tc.tile_pool(name="ps_s", bufs=2, space="PSUM") as ps_s,
            tc.tile_pool(name="ps_dn", bufs=1, space="PSUM") as ps_dn,
            tc.tile_pool(name="ps_x", bufs=1, space="PSUM") as ps_x,
            tc.tile_pool(name="ps_o", bufs=1, space="PSUM") as ps_o,
        ):
            # Weights: fp8 DoubleRow layout for q/k, bf16 [fi_part 128,
            # fi_chunk 4, fo 512] for v/p.
            w_tiles = {}
            for name, t in (("wq", wq), ("wk", wk)):
                wt = consts.tile([128, 2, 2, DIM], FP8, tag=f"w_{name}")
                nc.sync.dma_start(out=wt, in_=t.ap())
                w_tiles[name] = wt
            for name, t in (("wv", wv), ("wp", wp)):
                wt = consts.tile([128, 4, DIM], BF16, tag=f"w_{name}")
                nc.sync.dma_start(
                    out=wt, in_=t.ap().rearrange("(c p) f -> p c f", p=128)
                )
                w_tiles[name] = wt

            for tt_i in range(n_tt):
                t0 = tt_i * TT
                # ---- transposed input loads (host pre-transposed) ----
                qT = pin.tile([128, 4, TT], FP8, tag="qT")
                kT = pin.tile([128, 4, TT], FP8, tag="kT")
                vT = pin.tile([128, 4, TT], BF16, tag="vT")
                nc.sync.dma_start(out=qT, in_=qa[tt_i])
                nc.sync.dma_start(out=kT, in_=ka[tt_i])
                nc.sync.dma_start(out=vT, in_=va[tt_i])
                msel_t = pin.tile([128, PAIRS, 2, 128], BF16, tag="msel")
                nc.sync.dma_start(out=msel_t, in_=msel_d.ap()[tt_i])
                m01_t = pin.tile([128, PAIRS], F32, tag="m01")
                nc.sync.dma_start(out=m01_t, in_=m01_d.ap()[tt_i])

                # ---- v projection -> masked token-major bf16 ----
                # (first so its ACT copies don't delay exp; vh rows of
                # masked tokens are zeroed via the per-partition copy scale
                # = numerator side of the masked softmax)
                yv = py.tile([128, PAIRS, DIM], BF16, tag="yv")
                for ct in range(PAIRS):
                    ps = ps_pj.tile([128, DIM], F32, tag="pj")
                    for ci in range(4):
                        nc.tensor.matmul(
                            ps,
                            lhsT=vT[:, ci, ts(ct, 128)],
                            rhs=w_tiles["wv"][:, ci, :],
                            start=(ci == 0),
                            stop=(ci == 3),
                        )
                    nc.scalar.activation(
                        out=yv[:, ct, :],
                        in_=ps,
                        func=mybir.ActivationFunctionType.Copy,
                        scale=m01_t[:, ds(ct, 1)],
                    )
                # second window's vh rows relocated to partition base 0
                yv_hi = py.tile([64, PAIRS, DIM], BF16, tag="yv_hi")
                nc.gpsimd.dma_start(out=yv_hi, in_=yv[64:128, :, :])

                # ---- k projection -> zero-padded per-head-slot layout ----
                # ykTz[p<64, c, pr, w, 0, tk] = head-2c rows, ykTz[p>=64, c,
                # pr, w, 1, tk] = head-2c+1 rows, other slots zero. The
                # (slot, tk) tail is contiguous, so ykTz[:, c, pr, w] is the
                # block-diag stationary operand of an S.T matmul. Zero slots
                # are never overwritten, so memset only on each pool
                # buffer's first use.
                ykTz = py.tile([128, 4, PAIRS, 2, 2, N], BF16, tag="ykTz")
                if tt_i < 2:
                    nc.gpsimd.memset(ykTz[0:64, :, :, :, 1, :], 0.0)
                    nc.gpsimd.memset(ykTz[64:128, :, :, :, 0, :], 0.0)
                for co in range(4):
                    ps = ps_pj.tile([128, TT], F32, tag="pj")
                    for kp in range(2):
                        nc.tensor.matmul(
                            ps,
                            lhsT=w_tiles["wk"][:, kp, :, ts(co, 128)],
                            rhs=kT[:, ds(2 * kp, 2), :],
                            start=(kp == 0),
                            stop=(kp == 1),
                            perf_mode=mybir.MatmulPerfMode.DoubleRow,
                        )
                    psw = ps.rearrange("p (r w t) -> p r w t", r=PAIRS, w=2)
                    # split the zero-pad copies across ACT and DVE so the
                    # S.T gate (ykTz ready) clears ~2x sooner
                    if co < 2:
                        nc.scalar.mul(
                            out=ykTz[0:64, co, :, :, 0, :],
                            in_=psw[0:64],
                            mul=1.0 / SK,
                        )
                        nc.scalar.mul(
                            out=ykTz[64:128, co, :, :, 1, :],
                            in_=psw[64:128],
                            mul=1.0 / SK,
                        )
                    else:
                        nc.vector.tensor_scalar_mul(
                            ykTz[0:64, co, :, :, 0, :], psw[0:64], 1.0 / SK
                        )
                        nc.vector.tensor_scalar_mul(
                            ykTz[64:128, co, :, :, 1, :], psw[64:128], 1.0 / SK
                        )

                # ---- q projection (fp8 DoubleRow, K=256 per MM) ----
                # -> feature-major bf16 [fo 128, c, tok]; copies on DVE,
                # 1/SQ undoes the fp8 weight pre-scale.
                yqT = py.tile([128, 4, TT], BF16, tag="yqT")
                for co in range(4):
                    ps = ps_pj.tile([128, TT], F32, tag="pj")
                    for kp in range(2):
                        nc.tensor.matmul(
                            ps,
                            lhsT=w_tiles["wq"][:, kp, :, ts(co, 128)],
                            rhs=qT[:, ds(2 * kp, 2), :],
                            start=(kp == 0),
                            stop=(kp == 1),
                            perf_mode=mybir.MatmulPerfMode.DoubleRow,
                        )
                    nc.vector.tensor_scalar_mul(yqT[:, co, :], ps, 1.0 / SQ)

                # ---- attention, software-pipelined over 128-token pairs ----
                def stage_a(pr):
                    """S.T for all 8 heads + exp + odd-parity relocation."""
                    base = pr * 128
                    spT = ps_s.tile([128, 4, 2, N], F32, tag="spT")
                    for ch in range(4):
                        for w in range(2):
                            nc.tensor.matmul(
                                spT[:, ch, w, :],
                                lhsT=ykTz[:, ch, pr, w, :, :],
                                rhs=yqT[:, ch, ds(base + 64 * w, 64)],
                                start=True,
                                stop=True,
                            )
                    pexpT = pp.tile([128, 4, 2, N], BF16, tag="pexpT")
                    nc.scalar.activation(
                        out=pexpT, in_=spT, func=mybir.ActivationFunctionType.Exp
                    )
                    pexpT_od = pp.tile([64, 4, 2, N], BF16, tag="pexpT_od")
                    nc.gpsimd.dma_start(out=pexpT_od, in_=pexpT[64:128, :, :, :])
                    return pexpT, pexpT_od

                def stage_b1(pr, pexpT, pexpT_od):
                    """Masked softmax denominators -> broadcast reciprocals.

                    Runs in parallel with the (unnormalized) P@V matmuls;
                    normalization is linear so it is applied to X instead
                    of P. The M=128 duplicated-column msel makes the
                    denominators land already broadcast per parity block."""
                    dn = ps_dn.tile([128, 4, 2, N], F32, tag="dn")
                    for w in range(2):
                        nc.tensor.matmul(
                            dn[:, :, w, :],
                            lhsT=msel_t[:, pr, w, :],
                            rhs=pexpT[:, :, w, :],
                            start=True,
                            stop=True,
                        )
                    recb = pst.tile([128, 4, 2, N], F32, tag="recb")
                    nc.vector.reciprocal_approx_fast(
                        out=recb.rearrange("p a w t -> p (a w t)"),
                        in_=dn.rearrange("p a w t -> p (a w t)"),
                    )
                    return recb

                def stage_b2x(pr, pexpT, pexpT_od, recb):
                    """Unnormalized P@V, then normalize X by recb."""
                    xt = ps_x.tile([128, 4, 2, N], F32, tag="xt")
                    for h in range(HEADS):
                        ch, hh = h // 2, (h % 2) * 64
                        psrc = pexpT if h % 2 == 0 else pexpT_od
                        for w in range(2):
                            vsrc = yv if w == 0 else yv_hi
                            nc.tensor.matmul(
                                xt[ds(hh, 64), ch, w, :],
                                lhsT=vsrc[ds(0, 64), pr, ts(h, 64)],
                                rhs=psrc[ds(0, 64), ch, w, :],
                                start=True,
                                stop=True,
                            )
                    xts = pp.tile([128, 4, 2, N], BF16, tag="xts")
                    nc.vector.tensor_tensor(
                        out=xts, in0=xt, in1=recb, op=mybir.AluOpType.mult
                    )
                    return xts

                def stage_b2o(pr, xts):
                    """Output projection: [tok 128, fo 512]."""
                    base = pr * 128
                    po = ps_o.tile([128, DIM], F32, tag="po")
                    for c in range(4):
                        nc.tensor.matmul(
                            po,
                            lhsT=xts[:, c, :, :],
                            rhs=w_tiles["wp"][:, c, :],
                            start=(c == 0),
                            stop=(c == 3),
                        )
                    osb = pout.tile([128, DIM], BF16, tag="osb")
                    nc.vector.tensor_copy(out=osb, in_=po)
                    # gpsimd DMA queue: keeps the sync queue input-only
                    nc.gpsimd.dma_start(
                        out=oa[t0 + base : t0 + base + 128, :], in_=osb
                    )

                pa = [None] * PAIRS
                prec = [None] * PAIRS
                pxts = [None] * PAIRS
                pa[0] = stage_a(0)
                pa[1] = stage_a(1)
                prec[0] = stage_b1(0, *pa[0])
                pa[2] = stage_a(2)
                prec[1] = stage_b1(1, *pa[1])
                pxts[0] = stage_b2x(0, *pa[0], prec[0])
                pa[3] = stage_a(3)
                prec[2] = stage_b1(2, *pa[2])
                pxts[1] = stage_b2x(1, *pa[1], prec[1])
                stage_b2o(0, pxts[0])
                prec[3] = stage_b1(3, *pa[3])
                pxts[2] = stage_b2x(2, *pa[2], prec[2])
                stage_b2o(1, pxts[1])
                pxts[3] = stage_b2x(3, *pa[3], prec[3])
                stage_b2o(2, pxts[2])
                stage_b2o(3, pxts[3])

    nc.compile()
    return nc


_PROGRAM_CACHE = {}


def _get_program(win_per_core):
    if win_per_core not in _PROGRAM_CACHE:
        _PROGRAM_CACHE[win_per_core] = build_program(win_per_core)
    return _PROGRAM_CACHE[win_per_core]


def _feature_major_tiles(x_flat):
    """[tok, 512] -> [n_tt, 128, 4, TT] so each T-tile block is one
    fully-contiguous transposed DMA read."""
    tok = x_flat.shape[0]
    n_tt = tok // TT
    xt = x_flat.reshape(n_tt, TT, 4, 128).transpose(0, 3, 2, 1)
    return np.ascontiguousarray(xt)


def _fp8(x):
    return np.clip(np.asarray(x, np.float32), -240.0, 240.0).astype(NP_FP8)


def _dr_weights(wT_scaled):
    """[512 fi, 512 fo] -> DoubleRow fp8 layout [ki 128, kp 2, ko 2, fo]."""
    w8 = _fp8(wT_scaled)
    return np.ascontiguousarray(
        w8.reshape(2, 2, 128, DIM).transpose(2, 0, 1, 3)
    )


def make_in_maps(q, k, v, mask, Wq, Wk, Wv, Wp, ncores=NCORES):
    """Host-side shard + layout prep. Returns list of per-core input dicts."""
    B, n, C = q.shape
    win_pc = B // ncores
    qf = np.ascontiguousarray(q.reshape(B * n, C))
    kf = np.ascontiguousarray(k.reshape(B * n, C))
    vf = np.ascontiguousarray(v.reshape(B * n, C))

    mflat = (mask != 0).astype(np.float32).reshape(B * n)

    wq_t = _dr_weights(Wq.T * (SCALE * SQ))
    wk_t = _dr_weights(Wk.T * SK)
    wv_t = np.ascontiguousarray(Wv.T.astype(NP_BF16))
    wp_t = np.ascontiguousarray(Wp.T.astype(NP_BF16))

    tok_pc = win_pc * n
    n_tt = tok_pc // TT
    in_maps = []
    for c in range(ncores):
        sl = slice(c * tok_pc, (c + 1) * tok_pc)
        mc = mflat[sl]
        # m01[tt, p, ct] = mask of token tt*512 + ct*128 + p
        m01 = np.ascontiguousarray(
            mc.reshape(n_tt, PAIRS, 128).transpose(0, 2, 1)
        )
        # msel[tt, p, pr, w, m]: parity selector rows carry the window
        # (pr, w) mask at tk = p % 64, duplicated over 64 m-columns per
        # parity so the denominator matmul output is partition-broadcast
        mwin = mc.reshape(n_tt, PAIRS, 2, 64).transpose(0, 3, 1, 2)[..., None]
        msel = np.zeros((n_tt, 128, PAIRS, 2, 128), NP_BF16)
        msel[:, 0:64, :, :, 0:64] = mwin
        msel[:, 64:128, :, :, 64:128] = mwin
        in_maps.append(
            {
                "qT": _feature_major_tiles(_fp8(qf[sl])),
                "kT": _feature_major_tiles(_fp8(kf[sl])),
                "vT": _feature_major_tiles(vf[sl].astype(NP_BF16)),
                "msel": msel,
                "m01": m01,
                "wq": wq_t,
                "wk": wk_t,
                "wv": wv_t,
                "wp": wp_t,
            }
        )
    return in_maps


def _reference_numpy(q, k, v, mask, Wq, bq, Wk, bk, Wv, bv, Wp, bp):
    """Full-precision host fallback (only used for nonzero bq/bk)."""
    B, n, C = q.shape
    qh = (q.reshape(-1, C) @ Wq.T + bq).reshape(B, n, HEADS, HD).transpose(0, 2, 1, 3)
    kh = (k.reshape(-1, C) @ Wk.T + bk).reshape(B, n, HEADS, HD).transpose(0, 2, 1, 3)
    vh = (v.reshape(-1, C) @ Wv.T + bv).reshape(B, n, HEADS, HD).transpose(0, 2, 1, 3)
    s = np.einsum("bhqd,bhkd->bhqk", qh * SCALE, kh)
    s = np.where((mask[:, None, None, :] == 0), np.float32(-10000.0), s)
    s = s - s.max(-1, keepdims=True)
    e = np.exp(s)
    p = e / e.sum(-1, keepdims=True)
    x = np.einsum("bhqk,bhkd->bhqd", p, vh)
    x = x.transpose(0, 2, 1, 3).reshape(B, n, C)
    return (x @ Wp.T + bp).astype(np.float32)


def kernel(q, k, v, mask, Wq, bq, Wk, bk, Wv, bv, Wp, bp, trace=False):
    q = np.asarray(q, np.float32)
    k = np.asarray(k, np.float32)
    v = np.asarray(v, np.float32)
    mask = np.asarray(mask)
    Wq, Wk, Wv, Wp = (np.asarray(w, np.float32) for w in (Wq, Wk, Wv, Wp))
    bq, bk, bv, bp = (np.asarray(b, np.float32) for b in (bq, bk, bv, bp))

    if np.any(bq) or np.any(bk):
        return _reference_numpy(q, k, v, mask, Wq, bq, Wk, bk, Wv, bv, Wp, bp)

    B, n, C = q.shape
    win_pc = B // NCORES
    nc = _get_program(win_pc)
    in_maps = make_in_maps(q, k, v, mask, Wq, Wk, Wv, Wp)
    res = run_bass_kernel_spmd(
        nc, in_maps, core_ids=list(range(NCORES)), trace=trace
    )
    outs = np.concatenate(
        [np.asarray(r["out"], np.float32) for r in res.results], axis=0
    )
    outs = outs.reshape(B, n, C)
    # bv flows through attention linearly (softmax rows sum to 1); with bp it
    # folds into a single output bias.
    bout = bp + bv @ Wp.T
    if np.any(bout):
        outs = outs + bout.astype(np.float32)
    if trace:
        kernel._last_result = res
    return outs
